# revision 70
# baseline (speedup 1.0000x reference)
"""BitTransformerBlock on 8 Trainium2 NeuronCores — v4 (~450us, was 675us).

Token-parallel sharding: the flattened (B*S)=4096 tokens are split 512 per
core; cores 0-3 hold batch 0, cores 4-7 batch 1.  Each core computes LN1 and
the q/k/v projections for its own tokens, in-kernel AllGathers (replica
groups [0..3], [4..7]) share K and V across each batch group, and everything
downstream (attention over the full 2048-token context, out-proj, LN2, the
quantized FFN) is token-local.

Structure (v4):
  * the K/V exchange is TWO fp8e4m3 AllGathers (K right after the k
    projection, V after v) — ring time is linear in bytes (~25us/0.5MB
    over the 4-core group), consecutive collectives serialize, and the
    first collective pays a one-time ~15us trigger latency after the
    runtime's init barrier (which itself ends at 30-75us, jittery).
    Scores run as soon as K lands; V hides under the 4-head-pair scores
    prologue + junk matmuls.
  * the whole attention block runs in fp8e4m3: q/k/v cast to fp8 at the
    projection PSUM drain, fp8 scores matmuls (quadrant-packed head
    pairs), exp produced in fp8 (ACT exact exp for 8/16 tiles, DVE
    int8-Schraudolph bitcast for 8/16 — together they outrun the PE), and
    AV via fp8 DoubleRow matmuls contracting TWO key chunks per
    instruction.  V carries a ones column so the AV matmul emits the
    softmax denominator.
  * softmax normalization: ACT drains numerators/denominator rows (plain
    Copies, no activation tables), a PE outer-product broadcasts the
    denominator across partitions, and 1/d is a DVE bit-trick seed + one
    Newton step on the Pool engine.  (DVE's reciprocal instruction is
    free-dim-serial ~3.3us/row; GpSimd partition_broadcast is several us;
    ACT Ln<->Exp swaps cost ~1.3us per table load — all avoided.)
  * layernorm rstd = bit-trick rsqrt + Newton on [128,1] DVE tiles, so the
    only activation-table loads in the kernel are Exp (attention) and
    Gelu (ffn1).
  * ffn2 runs two passes over the w2 chunks (token halves 01 then 23,
    chunk order rotated) so the first output drains + DMAs overlap the
    second pass's matmuls.

Precision: dense PE matmuls in bf16 with fp32 PSUM accumulation; attention
in fp8e4m3 (measured ~5.3e-3 absmax-rel vs the fp32 reference, gate 2e-2).
The BitNet FFN quantization rounding is exact (magic-number round in fp32);
softmax is computed without max subtraction (logits are small here).

Known timing facts (trace-verified): PE 512-row matmul ~213ns at full
clock + 173ns SBUF access latency (hidden only by back-to-back overlap);
PE p-states 0.65/1.2/2.4GHz ramp with continuous activity (HAM re-throttles
after ~3.4us idle — hence the junk-matmul warmers); ACT/DVE exp tiles
[128,2,512] cost ~1.1/1.2us each; cross-engine chains involving DMA hops
cost ~8us latency per hop (never put them on the critical path).
"""

import numpy as np
import ml_dtypes

import concourse.bacc as bacc
import concourse.bass as bass
import concourse.bass_isa as bass_isa
import concourse.mybir as mybir
import concourse.tile as tile
from concourse import masks
from concourse.bass_interp import get_hw_module
from concourse.bass_utils import run_bass_kernel_spmd

F32 = mybir.dt.float32
BF16 = mybir.dt.bfloat16
FP8 = mybir.dt.float8e4
INT8 = mybir.dt.int8
AF = mybir.ActivationFunctionType
OP = mybir.AluOpType
DR = mybir.MatmulPerfMode.DoubleRow

N_CORES = 8
B, S, D, H, FF = 2, 2048, 1024, 16, 4096
HD = D // H                 # 64
NTOK = B * S                # 4096
TOK = NTOK // N_CORES       # 512 tokens per core
TCH = TOK // 128            # 4 token chunks per core
DCH = D // 128              # 8
FFCH = FF // 128            # 32
NKC = S // 128              # 16 key chunks per batch
GROUPS = [[0, 1, 2, 3], [4, 5, 6, 7]]
CORES_PER_B = 4
EPS = 1e-5
MAGIC = 12582912.0          # 1.5 * 2**23: fp32 round-to-nearest-even trick
INV_SQRT_HD = 1.0 / 8.0
VSLOT = HD + 1              # 65: per-head V slot width (64 dims + ones col)

# Schraudolph-style exp on DVE, fp8e4m3 flavor:
# e^(x/8) ~= bitcast_fp8e4(int8(x*EXPA8 + EXPB8)).  fp8e4m3 bits are
# [sign|4exp|3mant]; int(8*(z+7)) ~= 2^z with small sawtooth error that is
# scale-invariant under softmax and measured negligible on the final output.
EXPA8 = 8.0 / (8.0 * float(np.log(2.0)))
EXPB8 = 7.0 * 8.0 - 0.477
# which of the 16 (group, head) exp tiles per head pair go to DVE instead
# of ACT: 8/8 keeps both engines just under the PE's fp8 rate so the PE
# never stalls on softmax.  (GpSimd cannot read PSUM, so it can't help.)
EXP_DVE = {1, 3, 5, 7, 9, 11, 13, 15}
EXP_POOL = frozenset()
# fp32 reciprocal magic (Newton seed): bitcast(RCP_MAGIC - bits(x)) ~ 1/x
# with ~3.4% sawtooth error; one Newton step brings it to ~0.1%.
RCP_MAGIC = float(0x7EF127EA)
# fp32 rsqrt magic: bitcast(RSQRT_MAGIC - (bits(x) >> 1)) ~ 1/sqrt(x)
RSQRT_MAGIC = float(0x5F3759DF)
JUNK_START = 10             # junk matmuls at t=0: warm the PE before inproj
JUNK_BRIDGE = 90            # junk matmuls bridging the K-gather wait


def build_program(s1, s2, biases, sim_gelu=False):
    """Emit the SPMD program.  `s1`/`s2` are the host-computed ternary weight
    scales; `biases` maps name -> bool for whether the tensor is non-trivial."""
    nc = bacc.Bacc("TRN2", target_bir_lowering=False, debug=False,
                   num_devices=N_CORES)

    # all layouts are p-major: [128, ...] with contiguous per-partition rows
    x_in = nc.dram_tensor("x_pm", [128, TCH, D], F32, kind="ExternalInput")
    wq_in = nc.dram_tensor("wqT", [128, DCH, D], BF16, kind="ExternalInput")
    wk_in = nc.dram_tensor("wkT", [128, DCH, D], BF16, kind="ExternalInput")
    wv_in = nc.dram_tensor("wvT", [128, DCH, D], BF16, kind="ExternalInput")
    wo_in = nc.dram_tensor("woT", [128, DCH, D], BF16, kind="ExternalInput")
    w1_in = nc.dram_tensor("w1T", [8, 128, DCH, 512], BF16, kind="ExternalInput")
    w2_in = nc.dram_tensor("w2T", [128, FFCH, D], BF16, kind="ExternalInput")
    out_d = nc.dram_tensor("out", [128, TCH, D], F32, kind="ExternalOutput")

    ext = {}
    for name, shape in [("ln1_g", [D]), ("ln1_b", [D]),
                        ("ln2_g", [D]), ("ln2_b", [D]),
                        ("in_bq", [128, DCH]), ("in_bk", [128, DCH]),
                        ("in_bv", [D]), ("out_b", [D]),
                        ("b1", [128, 8, 4]), ("b2", [D])]:
        key = {"in_bq": "in_proj_b", "in_bk": "in_proj_b",
               "in_bv": "in_proj_b", "out_b": "out_proj_b"}.get(name, name)
        if biases[key]:
            ext[name] = nc.dram_tensor(name, shape, F32, kind="ExternalInput")

    with tile.TileContext(nc) as tc:
        _emit(nc, tc, x_in, wq_in, wk_in, wv_in, wo_in, w1_in, w2_in,
              out_d, ext, s1, s2, biases, sim_gelu)
    nc.compile()
    return nc


def _scope(nc, name):
    sid = nc.enter_named_scope(name, False)
    return (name, sid[0] if isinstance(sid, tuple) else sid)


def _unscope(nc, tok):
    nc.leave_named_scope(tok[0], tok[1], False)


def _emit(nc, tc, x_in, wq_in, wk_in, wv_in, wo_in, w1_in, w2_in,
          out_d, ext, s1, s2, biases, sim_gelu=False):
    gelu_func = AF.Tanh if sim_gelu else AF.Gelu
    from contextlib import ExitStack

    es_top = ExitStack()
    dram = es_top.enter_context(tc.tile_pool(name="dram", bufs=1, space="DRAM"))
    const = es_top.enter_context(tc.tile_pool(name="const", bufs=1))
    stats = es_top.enter_context(tc.tile_pool(name="stats", bufs=4))

    # split fp8 bounce buffers: kT (8*512) and v 65-slot layout (4*16*65)
    KV_K = DCH * 512                      # 4096
    KV_V = TCH * H * VSLOT               # 4160
    k_bounce = dram.tile([128, KV_K], FP8)
    k_all = dram.tile([CORES_PER_B, 128, KV_K], FP8)
    v_bounce = dram.tile([128, KV_V], FP8)
    v_all = dram.tile([CORES_PER_B, 128, KV_V], FP8)

    eps_t = const.tile([128, 1], F32)
    nc.vector.memset(eps_t[:], EPS)
    magic_t = const.tile([128, 1], F32)
    nc.vector.memset(magic_t[:], MAGIC)
    id_f32 = const.tile([128, 128], F32)
    masks.make_identity(nc, id_f32[:])
    id_bf = const.tile([128, 128], BF16)
    nc.vector.tensor_copy(out=id_bf[:], in_=id_f32[:])
    junk_sb = const.tile([128, 512], BF16, tag="junk")
    nc.gpsimd.memset(junk_sb[:], 0.001)
    ones64 = const.tile([1, 64], BF16, tag="ones64")
    nc.vector.memset(ones64[:], 1.0)

    # (no warm-up collective: with the PE pre-warmed the k bounce data is
    # ready before the init barrier ends, so the K gather pays the one-time
    # ~15us trigger latency anyway and a warm-up only adds serialization)

    # broadcast tiles for non-trivial per-feature constants (token-major use)
    def load_row_bcast(name, width):
        """DRAM [width] -> sbuf [128, width] broadcast over partitions."""
        row = const.tile([1, width], F32, tag=f"row_{name}")
        nc.sync.dma_start(out=row[:], in_=ext[name][:].unsqueeze(0))
        t = const.tile([128, width], F32, tag=f"bc_{name}")
        nc.gpsimd.partition_broadcast(t[:], row[:], channels=128)
        return t

    def load_pm(name, w=DCH):
        t = const.tile([128, w], F32, tag=f"pm_{name}")
        nc.sync.dma_start(out=t[:], in_=ext[name][:])
        return t

    g1_bc = load_row_bcast("ln1_g", D) if biases["ln1_g"] else None
    b1ln_bc = load_row_bcast("ln1_b", D) if biases["ln1_b"] else None
    bq_pm = load_pm("in_bq") if biases["in_proj_b"] else None
    bk_pm = load_pm("in_bk") if biases["in_proj_b"] else None
    bv_bc = load_row_bcast("in_bv", D) if biases["in_proj_b"] else None
    bo_bc = load_row_bcast("out_b", D) if biases["out_proj_b"] else None
    g2_bc = load_row_bcast("ln2_g", D) if biases["ln2_g"] else None
    b2ln_bc = load_row_bcast("ln2_b", D) if biases["ln2_b"] else None
    bf2_bc = load_row_bcast("b2", D) if biases["b2"] else None

    # ---- pool nesting (stack allocator: LIFO lifetimes) -------------------
    es_D = ExitStack()
    pD = es_D.enter_context(tc.tile_pool(name="pD", bufs=1))    # x2
    es_A = ExitStack()
    pA = es_A.enter_context(tc.tile_pool(name="pA", bufs=1))    # x_sb
    es_C = ExitStack()
    pC = es_C.enter_context(tc.tile_pool(name="pC", bufs=1))    # oT, wo
    es_B = ExitStack()
    pB = es_B.enter_context(tc.tile_pool(name="pB", bufs=1))    # qT, KT, Vaug
    es_X = ExitStack()
    pX = es_X.enter_context(tc.tile_pool(name="pX", bufs=1))    # xT,nxT,wqkv

    x_sb = pA.tile([128, TCH, D], F32, tag="x")
    for t in range(TCH):
        nc.sync.dma_start(out=x_sb[:, t, :], in_=x_in[:, t, :])
    nxT = pX.tile([128, DCH, TOK], BF16, tag="nxT")
    wk_sb = pX.tile([128, DCH, D], BF16, tag="wk")
    nc.sync.dma_start(out=wk_sb[:], in_=wk_in[:])
    wv_sb = pX.tile([128, DCH, D], BF16, tag="wv")
    nc.sync.dma_start(out=wv_sb[:], in_=wv_in[:])
    wq_sb = pX.tile([128, DCH, D], BF16, tag="wq")
    nc.sync.dma_start(out=wq_sb[:], in_=wq_in[:])

    es_pre = ExitStack()
    ps_pre = es_pre.enter_context(tc.tile_pool(name="ps_pre", bufs=4,
                                               space="PSUM"))
    ps_mr_pool = es_pre.enter_context(tc.tile_pool(name="ps_mr", bufs=2,
                                                   space="PSUM"))
    s_pre = es_pre.enter_context(tc.tile_pool(name="s_pre", bufs=2))

    # warm the PE (HAM clock gate) with junk matmuls while x/weights load
    junk_ps = ps_mr_pool.tile([128, 512], F32, tag="junk_ps")

    def emit_junk(n):
        for _ in range(n):
            nc.tensor.matmul(junk_ps[:], lhsT=junk_sb[:, 0:128],
                             rhs=junk_sb[:], start=True, stop=True)

    emit_junk(JUNK_START)

    # ---- stage 1: LN1 (token-major) + PE transpose to feature-major ------
    sc1 = _scope(nc, "ln1")

    def ln_stats(src_ap, m_out, r_out):
        """mean/rstd of src_ap [128, D] fp32 -> m_out/r_out [128, 1].

        rstd is a DVE bit-trick rsqrt + one Newton step (all [128,1] ops,
        ~0.1us each) — the previous exp(-0.5*ln(v)) on ACT swapped
        activation tables twice per call, ~2.5us of ACT_TABLE_LOAD."""
        st = stats.tile([128, 2, 6], F32, tag="bnst")
        nc.vector.bn_stats(out=st[:, 0, :], in_=src_ap[:, 0:512])
        nc.vector.bn_stats(out=st[:, 1, :], in_=src_ap[:, 512:1024])
        mv = stats.tile([128, 2], F32, tag="mv")
        nc.vector.bn_aggr(out=mv[:], in_=st[:])
        nc.vector.tensor_copy(out=m_out, in_=mv[:, 0:1])
        d = stats.tile([128, 1], F32, tag="vd")
        nc.vector.tensor_scalar_add(out=d[:], in0=mv[:, 1:2], scalar1=EPS)
        yi = stats.tile([128, 1], mybir.dt.int32, tag="yi")
        nc.vector.tensor_scalar(out=yi[:], in0=d[:].bitcast(mybir.dt.int32),
                                scalar1=1, scalar2=None,
                                op0=OP.logical_shift_right)
        nc.vector.tensor_scalar(out=yi[:], in0=yi[:], scalar1=-1,
                                scalar2=RSQRT_MAGIC, op0=OP.mult, op1=OP.add)
        y0 = yi[:].bitcast(F32)
        e = stats.tile([128, 1], F32, tag="e1")
        nc.vector.tensor_tensor(out=e[:], in0=d[:], in1=y0, op=OP.mult)
        nc.vector.tensor_tensor(out=e[:], in0=e[:], in1=y0, op=OP.mult)
        nc.vector.tensor_scalar(out=e[:], in0=e[:], scalar1=-0.5,
                                scalar2=1.5, op0=OP.mult, op1=OP.add)
        nc.vector.tensor_tensor(out=r_out, in0=y0, in1=e[:], op=OP.mult)

    es_t = ExitStack()
    ps_t = es_t.enter_context(tc.tile_pool(name="ps_t", bufs=2, space="PSUM"))
    for t in range(TCH):
        m = stats.tile([128, 1], F32, tag="m1")
        r = stats.tile([128, 1], F32, tag="r1")
        ln_stats(x_sb[:, t, :], m[:], r[:])
        nx = s_pre.tile([128, D], BF16, tag="nx")
        nc.vector.tensor_scalar(out=nx[:], in0=x_sb[:, t, :], scalar1=m[:],
                                scalar2=r[:], op0=OP.subtract, op1=OP.mult)
        if g1_bc is not None:
            nc.vector.tensor_tensor(out=nx[:], in0=nx[:], in1=g1_bc[:],
                                    op=OP.mult)
        if b1ln_bc is not None:
            nc.vector.tensor_tensor(out=nx[:], in0=nx[:], in1=b1ln_bc[:],
                                    op=OP.add)
        for dg in range(2):
            pst = ps_t.tile([128, 4, 128], BF16, tag="pst1")
            for i in range(4):
                dc = 4 * dg + i
                nc.tensor.transpose(pst[:, i, :],
                                    nx[:, dc * 128:(dc + 1) * 128], id_bf[:])
            for i in range(4):
                dc = 4 * dg + i
                dst = nxT[:, dc, t * 128:(t + 1) * 128]
                if i % 2 == 0:
                    nc.vector.tensor_copy(out=dst, in_=pst[:, i, :])
                else:
                    nc.scalar.activation(out=dst, in_=pst[:, i, :],
                                         func=AF.Copy)
    es_t.close()
    _unscope(nc, sc1)

    # ---- stage 2: in_proj (k -> gather, then q, then v -> gather) --------
    sc2 = _scope(nc, "inproj")
    # k projection, feature-major fp8: kT[f, t]; two fo chunks are packed
    # per bounce DMA so each transfer moves 1KB/partition
    kc = None
    for fo in range(DCH):
        ps = ps_pre.tile([128, TOK], F32, tag="ps")
        for dc in range(DCH):
            nc.tensor.matmul(ps[:], lhsT=wk_sb[:, dc, fo * 128:(fo + 1) * 128],
                             rhs=nxT[:, dc, :], start=(dc == 0),
                             stop=(dc == DCH - 1))
        if fo % 2 == 0:
            kc = s_pre.tile([128, 2, TOK], FP8, tag="kc")
        if bk_pm is not None:
            nc.vector.tensor_scalar(out=kc[:, fo % 2, :], in0=ps[:],
                                    scalar1=bk_pm[:, fo:fo + 1], op0=OP.add)
        else:
            nc.vector.tensor_copy(out=kc[:, fo % 2, :], in_=ps[:])
        if fo % 2 == 1:
            nc.sync.dma_start(
                out=k_bounce[:, (fo - 1) * 512:(fo + 1) * 512], in_=kc[:])
    nc.gpsimd.collective_compute(
        "AllGather", OP.bypass, replica_groups=GROUPS,
        ins=[k_bounce.opt()], outs=[k_all.opt()])

    # v projection, token-major fp8, written into the 65-slot bounce layout
    for t in range(TCH):
        vc = s_pre.tile([128, H, VSLOT], FP8, tag="vc")
        nc.vector.memset(vc[:, :, HD:VSLOT], 1.0)
        for f2 in range(2):
            ps = ps_pre.tile([128, 512], F32, tag="ps")
            for dc in range(DCH):
                nc.tensor.matmul(ps[:], lhsT=nxT[:, dc, t * 128:(t + 1) * 128],
                                 rhs=wv_sb[:, dc, f2 * 512:(f2 + 1) * 512],
                                 start=(dc == 0), stop=(dc == DCH - 1))
            dst = vc[:, 8 * f2:8 * (f2 + 1), 0:HD]
            if bv_bc is not None:
                nc.vector.tensor_tensor(
                    out=dst, in0=ps[:].rearrange("p (h d) -> p h d", d=HD),
                    in1=bv_bc[:, f2 * 512:(f2 + 1) * 512].rearrange(
                        "p (h d) -> p h d", d=HD), op=OP.add)
            else:
                nc.vector.tensor_copy(
                    out=dst, in_=ps[:].rearrange("p (h d) -> p h d", d=HD))
        nc.sync.dma_start(
            out=v_bounce[:, t * H * VSLOT:(t + 1) * H * VSLOT].rearrange(
                "p (h v) -> p h v", v=VSLOT), in_=vc[:])
    nc.gpsimd.collective_compute(
        "AllGather", OP.bypass, replica_groups=GROUPS,
        ins=[v_bounce.opt()], outs=[v_all.opt()])

    # q projection, feature-major fp8 (after k/v so the gathers start early)
    qT_sb = pB.tile([128, DCH, TOK], FP8, tag="qT")
    for fo in range(DCH):
        ps = ps_pre.tile([128, TOK], F32, tag="ps")
        for dc in range(DCH):
            nc.tensor.matmul(ps[:], lhsT=wq_sb[:, dc, fo * 128:(fo + 1) * 128],
                             rhs=nxT[:, dc, :], start=(dc == 0),
                             stop=(dc == DCH - 1))
        if bq_pm is not None:
            nc.vector.tensor_scalar(out=qT_sb[:, fo, :], in0=ps[:],
                                    scalar1=bq_pm[:, fo:fo + 1], op0=OP.add)
        else:
            nc.vector.tensor_copy(out=qT_sb[:, fo, :], in_=ps[:])
    _unscope(nc, sc2)

    sc3 = _scope(nc, "unpack")
    # unpack gathered K^T / V(+ones) into SBUF — pure contiguous copies.
    # KT first: scores only need K, so they can start while V still gathers.
    KT = pB.tile([128, CORES_PER_B, DCH, TOK], FP8, tag="KT")
    Vaug = pB.tile([128, CORES_PER_B, TCH, H, VSLOT], FP8, tag="Va")
    for c in range(CORES_PER_B):
        nc.sync.dma_start(
            out=KT[:, c, :, :],
            in_=k_all[c][:].rearrange("p (d t) -> p d t", t=TOK))
    for c in range(CORES_PER_B):
        nc.sync.dma_start(
            out=Vaug[:, c, :, :, :],
            in_=v_all[c][:].rearrange("p (t h v) -> p t h v", h=H, v=VSLOT))

    # prefetch wo and the first FFN weight chunks while the collectives
    # run — issued here so no later DMA queues behind the attention-era
    # denominator traffic
    wo_sb = pC.tile([128, DCH, D], BF16, tag="wo")
    nc.sync.dma_start(out=wo_sb[:], in_=wo_in[:])
    w1t = [pD.tile([128, DCH, 512], BF16, tag=f"w1_{i}", name=f"w1_{i}")
           for i in range(2)]
    nc.sync.dma_start(out=w1t[0][:], in_=w1_in[0])
    nc.sync.dma_start(out=w1t[1][:], in_=w1_in[1])
    _unscope(nc, sc3)

    es_pre.close()
    es_X.close()

    # ---- stage 3: attention (software-pipelined over head pairs) ---------
    sc4 = _scope(nc, "attn")
    oT = pC.tile([128, DCH, TOK], BF16, tag="oT")

    es_5 = ExitStack()
    ps_s = es_5.enter_context(tc.tile_pool(name="ps_s", bufs=3, space="PSUM"))
    ps_av = es_5.enter_context(tc.tile_pool(name="ps_av", bufs=2, space="PSUM"))
    s5e = es_5.enter_context(tc.tile_pool(name="s5e", bufs=30))
    s5d = es_5.enter_context(tc.tile_pool(name="s5d", bufs=3))

    NG = NKC // 2  # 8 score groups per head pair, 2 key chunks each

    def emit_scores_group(hp, g):
        """scores^T = K^T.T @ q^T for group g (row-packed head pair), + exp.

        exp tiles are spread over ACT (exact exp -> fp8), DVE and GpSimd
        (int8-Schraudolph bitcast to fp8) so the three engines together
        outrun the PE and it never stalls."""
        pss = [ps_s.tile([128, 2, 512], F32, tag="pss",
                         name=f"pss{hp}_{g}_{i}") for i in range(2)]
        for j in range(2):
            kc = 2 * g + j
            c, tcc = divmod(kc, TCH)
            ksl = KT[:, c, hp, tcc * 128:(tcc + 1) * 128]
            nc.tensor.matmul(pss[0][:, j, :], lhsT=ksl[0:64, :],
                             rhs=qT_sb[0:64, hp, :], start=True, stop=True,
                             tile_position=(0, 0))
            nc.tensor.matmul(pss[1][:, j, :], lhsT=ksl[64:128, :],
                             rhs=qT_sb[64:128, hp, :], start=True,
                             stop=True, tile_position=(64, 0))
        es = []
        for jh in range(2):
            i = 2 * g + jh
            if i in EXP_DVE or i in EXP_POOL:
                eng = nc.vector if i in EXP_DVE else nc.gpsimd
                ei = s5e.tile([128, 2, 512], INT8,
                              tag="expi" if i in EXP_DVE else "expp",
                              name=f"ei{hp}_{g}_{jh}")
                eng.tensor_scalar(out=ei[:], in0=pss[jh][:],
                                  scalar1=EXPA8, scalar2=EXPB8,
                                  op0=OP.mult, op1=OP.add)
                es.append(ei[:].bitcast(FP8))
            else:
                e = s5e.tile([128, 2, 512], FP8, tag="exp",
                             name=f"e{hp}_{g}_{jh}")
                nc.scalar.activation(out=e[:], in_=pss[jh][:], func=AF.Exp,
                                     scale=INV_SQRT_HD)
                es.append(e[:])
        return es

    AV_DOUBLE_ROW = True

    def emit_av_group(hp, g, pavs, exp_pair):
        """o^T[h] += V[2g:2g+2].T @ exp.  DoubleRow contracts both key
        chunks in one instruction but disables fast-weight-load; plain fp8
        uses two matmuls with FWL — measured A/B to pick."""
        c, tcc = divmod(2 * g, TCH)
        for jh in range(2):
            h = 2 * hp + jh
            if AV_DOUBLE_ROW:
                nc.tensor.matmul(pavs[jh][0:VSLOT, :],
                                 lhsT=Vaug[:, c, tcc:tcc + 2, h, :],
                                 rhs=exp_pair[jh],
                                 start=(g == 0), stop=(g == NG - 1),
                                 perf_mode=DR)
            else:
                for j in range(2):
                    nc.tensor.matmul(pavs[jh][0:VSLOT, :],
                                     lhsT=Vaug[:, c, tcc + j, h, :],
                                     rhs=exp_pair[jh][:, j, :],
                                     start=(g == 0 and j == 0),
                                     stop=(g == NG - 1 and j == 1))

    def finish_av_start(hp, pavs):
        """Drain the AV psum on ACT (plain Copies, no activation tables):
        numerators -> oT unnormalized, denominator rows -> bf16 staging."""
        rr = []
        for jh in range(2):
            drow = s5d.tile([1, 512], BF16, tag="drow")
            nc.scalar.activation(out=drow[:], in_=pavs[jh][64:65, :],
                                 func=AF.Copy)
            nc.scalar.activation(out=oT[jh * 64:jh * 64 + 64, hp, :],
                                 in_=pavs[jh][0:64, :], func=AF.Copy)
            rr.append(drow)
        return rr

    def finish_av_mul(hp, rr):
        """oT /= den: PE outer-product broadcasts the denominators over the
        partitions (~0.2us), DVE seeds 1/d with the bit-trick, and the
        Newton refinement runs on the otherwise-idle Pool engine."""
        dps = ps_av.tile([128, 512], F32, tag="pav", name=f"dps{hp}")
        for jh in range(2):
            nc.tensor.matmul(dps[jh * 64:jh * 64 + 64, :], lhsT=ones64[:],
                             rhs=rr[jh][:], start=True, stop=True,
                             tile_position=(0, jh * 64))
        y0 = s5d.tile([128, 512], mybir.dt.int32, tag="y0")
        nc.vector.tensor_scalar(out=y0[:], in0=dps[:].bitcast(mybir.dt.int32),
                                scalar1=-1, scalar2=RCP_MAGIC,
                                op0=OP.mult, op1=OP.add)
        y0f = y0[:].bitcast(F32)
        e2 = s5d.tile([128, 512], F32, tag="e2")
        nc.vector.tensor_tensor(out=e2[:], in0=dps[:], in1=y0f, op=OP.mult)
        nc.gpsimd.tensor_scalar(out=e2[:], in0=e2[:], scalar1=-1.0,
                                scalar2=2.0, op0=OP.mult, op1=OP.add)
        nc.gpsimd.tensor_tensor(out=oT[:, hp, :], in0=oT[:, hp, :],
                                in1=y0f, op=OP.mult)
        nc.gpsimd.tensor_tensor(out=oT[:, hp, :], in0=oT[:, hp, :],
                                in1=e2[:], op=OP.mult)

    # software pipeline: scores run three head pairs ahead of AV (the
    # depth-3 prologue covers the V-gather wait), with junk matmuls
    # topping up the PE queue until V lands
    NHP = H // 2
    DEPTH = 4

    def junk_bridge(n):
        for i in range(n):
            jt = ps_av.tile([128, 512], F32, tag="pav", name=f"junkb{i}")
            nc.tensor.matmul(jt[:], lhsT=junk_sb[:, 0:128], rhs=junk_sb[:],
                             start=True, stop=True)

    # junk first: it has no dependencies, so it fills the PE while the K
    # gather + unpack land (the prologue scores need KT)
    junk_bridge(JUNK_BRIDGE)
    exp_q = [[emit_scores_group(hp, g) for g in range(NG)]
             for hp in range(DEPTH)]

    pending = None
    for hp in range(NHP):
        # normalize of hp-1 is emitted at iteration start so the psum ring
        # rotates cleanly: pav0(h-1), pav1(h-1), dps(h-1), pav0(h), ...
        if pending is not None:
            finish_av_mul(*pending)
            pending = None
        pavs = [ps_av.tile([128, 512], F32, tag="pav", name=f"pav{hp}_{jh}")
                for jh in range(2)]
        nxt = [None] * NG
        for g in range(NG):
            if hp + DEPTH < NHP:
                nxt[g] = emit_scores_group(hp + DEPTH, g)
            emit_av_group(hp, g, pavs, exp_q[0][g])
        exp_q = exp_q[1:] + [nxt]
        pending = (hp, finish_av_start(hp, pavs))
    finish_av_mul(*pending)
    es_5.close()
    _unscope(nc, sc4)

    # ---- stage 4: out_proj + residual ------------------------------------
    sc6 = _scope(nc, "outproj")
    x2 = pD.tile([128, TCH, D], F32, tag="x2")
    es_6 = ExitStack()
    ps6 = es_6.enter_context(tc.tile_pool(name="ps6", bufs=4, space="PSUM"))
    for t in range(TCH):
        for f2 in range(2):
            ps = ps6.tile([128, 512], F32, tag="ps6")
            for dc in range(DCH):
                nc.tensor.matmul(ps[:], lhsT=oT[:, dc, t * 128:(t + 1) * 128],
                                 rhs=wo_sb[:, dc, f2 * 512:(f2 + 1) * 512],
                                 start=(dc == 0), stop=(dc == DCH - 1))
            dst = x2[:, t, f2 * 512:(f2 + 1) * 512]
            nc.vector.tensor_tensor(out=dst, in0=ps[:],
                                    in1=x_sb[:, t, f2 * 512:(f2 + 1) * 512],
                                    op=OP.add)
            if bo_bc is not None:
                nc.vector.tensor_tensor(out=dst, in0=dst,
                                        in1=bo_bc[:, f2 * 512:(f2 + 1) * 512],
                                        op=OP.add)
    es_6.close()
    es_B.close()
    es_C.close()
    es_A.close()
    _unscope(nc, sc6)

    # ---- stage 5: LN2 + act_quant (dq1 folded in) + PE transpose ---------
    # pF opens here (after the attention-era pools closed) so its large
    # tiles reuse the freed SBUF region
    es_F = ExitStack()
    pF = es_F.enter_context(tc.tile_pool(name="pF", bufs=1))
    sc7 = _scope(nc, "ln2q")
    hqT = pF.tile([128, DCH, TOK], BF16, tag="hqT")
    # w2 is streamed in 4 chunks of 8 fc each, double-buffered; first loads
    # issued here (they have until ffn2 to land)
    w2t = [pF.tile([128, 8, D], BF16, tag=f"w2_{i}", name=f"w2_{i}")
           for i in range(2)]
    nc.sync.dma_start(out=w2t[0][:], in_=w2_in[:, 0:8, :])
    nc.sync.dma_start(out=w2t[1][:], in_=w2_in[:, 8:16, :])

    es_7 = ExitStack()
    ps7 = es_7.enter_context(tc.tile_pool(name="ps7", bufs=2, space="PSUM"))
    s7 = es_7.enter_context(tc.tile_pool(name="s7", bufs=2))
    for t in range(TCH):
        m = stats.tile([128, 1], F32, tag="m2")
        r = stats.tile([128, 1], F32, tag="r2")
        ln_stats(x2[:, t, :], m[:], r[:])
        # h = (x2 - m) * r on ACT: Identity(x2*r + (-m*r)), freeing DVE for
        # the absmax/quant chain
        mb = stats.tile([128, 1], F32, tag="mb")
        nc.vector.tensor_tensor(out=mb[:], in0=m[:], in1=r[:], op=OP.mult)
        nc.vector.tensor_scalar_mul(out=mb[:], in0=mb[:], scalar1=-1.0)
        h = s7.tile([128, D], F32, tag="h")
        nc.scalar.activation(out=h[:], in_=x2[:, t, :], func=AF.Identity,
                             scale=r[:], bias=mb[:])
        if g2_bc is not None:
            nc.vector.tensor_tensor(out=h[:], in0=h[:], in1=g2_bc[:],
                                    op=OP.mult)
        if b2ln_bc is not None:
            nc.vector.tensor_tensor(out=h[:], in0=h[:], in1=b2ln_bc[:],
                                    op=OP.add)
        # (no EPS clamp: h is a layernorm output, absmax over 1024 unit-var
        # features is always >> EPS)
        am = stats.tile([128, 1], F32, tag="am")
        nc.vector.tensor_reduce(out=am[:], in_=h[:], axis=mybir.AxisListType.X,
                                op=OP.max, apply_absolute_value=True)
        sc = stats.tile([128, 1], F32, tag="sc")
        nc.vector.reciprocal(out=sc[:], in_=am[:])
        nc.vector.tensor_scalar_mul(out=sc[:], in0=sc[:], scalar1=127.0)
        dq1 = stats.tile([128, 1], F32, tag="dq1")
        nc.vector.tensor_scalar_mul(out=dq1[:], in0=am[:],
                                    scalar1=float(s1) / 127.0)
        # the magic-round add runs on ACT (Copy applies scale*in + bias);
        # only the subtract-and-dequant stays on DVE
        rq = s7.tile([128, D], F32, tag="rq")
        nc.scalar.activation(out=rq[:], in_=h[:], func=AF.Copy,
                             scale=sc[:], bias=float(MAGIC))
        hq = s7.tile([128, D], BF16, tag="hq")
        nc.vector.tensor_scalar(out=hq[:], in0=rq[:], scalar1=magic_t[:],
                                scalar2=dq1[:], op0=OP.subtract, op1=OP.mult)
        for dg in range(2):
            pst = ps7.tile([128, 4, 128], BF16, tag="pst")
            for i in range(4):
                dc = 4 * dg + i
                nc.tensor.transpose(pst[:, i, :],
                                    hq[:, dc * 128:(dc + 1) * 128], id_bf[:])
            for i in range(4):
                dc = 4 * dg + i
                dst = hqT[:, dc, t * 128:(t + 1) * 128]
                if i % 2 == 0:
                    nc.vector.tensor_copy(out=dst, in_=pst[:, i, :])
                else:
                    nc.scalar.activation(out=dst, in_=pst[:, i, :],
                                         func=AF.Copy)
    es_7.close()
    _unscope(nc, sc7)

    # ---- stage 6: FFN mm1 (transposed: y1T = w1 @ hqT) + gelu ------------
    sc8 = _scope(nc, "ffn1")
    y1g = pF.tile([128, FFCH, TOK], BF16, tag="y1g")
    run = pF.tile([128, TOK], F32, tag="runmax")
    nc.vector.memset(run[:], 0.0)
    if biases["b1"]:
        b1_pm = const.tile([128, 8, 4], F32, tag="pm_b1")
        nc.sync.dma_start(out=b1_pm[:], in_=ext["b1"][:])

    es_8 = ExitStack()
    ps8 = es_8.enter_context(tc.tile_pool(name="ps8", bufs=4, space="PSUM"))
    for ffo in range(8):
        wt = w1t[ffo % 2]
        for fo2 in range(4):
            fc = 4 * ffo + fo2
            ps = ps8.tile([128, TOK], F32, tag="ps8")
            for dc in range(DCH):
                nc.tensor.matmul(ps[:], lhsT=wt[:, dc, fo2 * 128:(fo2 + 1) * 128],
                                 rhs=hqT[:, dc, :], start=(dc == 0),
                                 stop=(dc == DCH - 1))
            if biases["b1"]:
                nc.scalar.activation(out=y1g[:, fc, :], in_=ps[:],
                                     func=gelu_func,
                                     bias=b1_pm[:, ffo, fo2:fo2 + 1])
            else:
                nc.scalar.activation(out=y1g[:, fc, :], in_=ps[:],
                                     func=gelu_func)
            # gelu(x) >= -0.17, and per-token max over 4096 features is
            # always >> 0.17 here, so plain max equals abs-max exactly
            nc.vector.tensor_tensor(out=run[:], in0=run[:], in1=y1g[:, fc, :],
                                    op=OP.max)
        # prefetch chunk ffo+2 into the buffer this ffo just finished reading
        if ffo + 2 < 8:
            nc.sync.dma_start(out=w1t[ffo % 2][:], in_=w1_in[ffo + 2])
    es_8.close()
    _unscope(nc, sc8)

    # ---- stage 7: act_quant of y1 (feature-major) + mm2 + residual -------
    sc9 = _scope(nc, "ffn2")
    # global per-token absmax: partition all-reduce, already broadcast
    # (no EPS clamp: the gelu absmax over 4096 features is always >> EPS)
    am_bc = pF.tile([128, TOK], F32, tag="am_bc")
    nc.gpsimd.partition_all_reduce(am_bc[:], run[:], channels=128,
                                   reduce_op=bass_isa.ReduceOp.max)
    sc_bc = pF.tile([128, TOK], F32, tag="sc_bc")
    nc.vector.reciprocal(out=sc_bc[:], in_=am_bc[:])
    nc.vector.tensor_scalar_mul(out=sc_bc[:], in0=sc_bc[:], scalar1=127.0)

    # keep the PE (and its HAM clock) busy while the absmax chain
    # (all_reduce -> reciprocal -> scale) resolves
    es_9j = ExitStack()
    ps9j = es_9j.enter_context(tc.tile_pool(name="ps9j", bufs=2, space="PSUM"))
    for i in range(30):
        jt = ps9j.tile([128, 512], F32, tag="jk", name=f"jk{i}")
        nc.tensor.matmul(jt[:], lhsT=junk_sb[:, 0:128], rhs=junk_sb[:],
                         start=True, stop=True)
    es_9j.close()

    es_9t = ExitStack()
    ps9t = es_9t.enter_context(tc.tile_pool(name="ps9t", bufs=1, space="PSUM"))
    one_t = const.tile([1, 1], F32, tag="one")
    nc.vector.memset(one_t[:], 1.0)
    dq2 = const.tile([128, TCH], F32, tag="dq2")
    pst2 = ps9t.tile([128, TCH], F32, tag="pst2")
    for t in range(TCH):
        nc.tensor.matmul(pst2[:, t:t + 1],
                         lhsT=am_bc[0:1, t * 128:(t + 1) * 128],
                         rhs=one_t[:], start=True, stop=True)
    nc.vector.tensor_scalar_mul(out=dq2[:], in0=pst2[:],
                                scalar1=float(s2) / 127.0)
    es_9t.close()

    y1qT = pF.tile([128, FFCH, TOK], BF16, tag="y1qT")
    es_9 = ExitStack()
    ps9 = es_9.enter_context(tc.tile_pool(name="ps9", bufs=8, space="PSUM"))
    s9 = es_9.enter_context(tc.tile_pool(name="s9", bufs=3))
    psf = [ps9.tile([128, 512], F32, tag="ps9", name=f"ps9_{t}_{f2}")
           for t in range(TCH) for f2 in range(2)]
    def drain_out(t, f2):
        outt = s9.tile([128, 512], F32, tag="outt")
        nc.vector.scalar_tensor_tensor(
            out=outt[:], in0=psf[2 * t + f2][:], scalar=dq2[:, t:t + 1],
            in1=x2[:, t, f2 * 512:(f2 + 1) * 512], op0=OP.mult, op1=OP.add)
        if bf2_bc is not None:
            nc.vector.tensor_tensor(
                out=outt[:], in0=outt[:],
                in1=bf2_bc[:, f2 * 512:(f2 + 1) * 512], op=OP.add)
        nc.sync.dma_start(out=out_d[:, t, f2 * 512:(f2 + 1) * 512],
                          in_=outt[:])

    # two passes over fc: token half 0/1 first (with the quant), then
    # halves 2/3 — so the first drains + output DMAs overlap the second
    # pass's matmuls instead of all landing after the last fc
    for fc in range(FFCH):
        tmp = s9.tile([128, TOK], F32, tag="qtmp")
        nc.vector.tensor_tensor(out=tmp[:], in0=y1g[:, fc, :], in1=sc_bc[:],
                                op=OP.mult)
        nc.vector.tensor_scalar(out=y1qT[:, fc, :], in0=tmp[:], scalar1=MAGIC,
                                scalar2=MAGIC, op0=OP.add, op1=OP.subtract)
        w2c, fci = divmod(fc, 8)
        for t in range(2):
            for f2 in range(2):
                nc.tensor.matmul(psf[2 * t + f2][:],
                                 lhsT=y1qT[:, fc, t * 128:(t + 1) * 128],
                                 rhs=w2t[w2c % 2][:, fci, f2 * 512:(f2 + 1) * 512],
                                 start=(fc == 0), stop=(fc == FFCH - 1))
        if fci == 7 and w2c + 2 < 4:
            nc.sync.dma_start(out=w2t[w2c % 2][:],
                              in_=w2_in[:, 8 * (w2c + 2):8 * (w2c + 3), :])
    for t in range(2):
        for f2 in range(2):
            drain_out(t, f2)
    # pass 2 consumes the still-resident w2 chunks (2, 3) first, then
    # re-streams chunks 0 and 1 into the buffers they free (fp32 psum
    # accumulation order over fc is commutative)
    seq = list(range(16, 32)) + list(range(0, 16))
    for si, fc in enumerate(seq):
        w2c, fci = divmod(fc, 8)
        for t in range(2, TCH):
            for f2 in range(2):
                nc.tensor.matmul(psf[2 * t + f2][:],
                                 lhsT=y1qT[:, fc, t * 128:(t + 1) * 128],
                                 rhs=w2t[w2c % 2][:, fci, f2 * 512:(f2 + 1) * 512],
                                 start=(si == 0), stop=(si == len(seq) - 1))
        if fci == 7 and w2c >= 2:
            nxt = w2c - 2
            nc.sync.dma_start(out=w2t[nxt % 2][:],
                              in_=w2_in[:, 8 * nxt:8 * (nxt + 1), :])
    for t in range(2, TCH):
        for f2 in range(2):
            drain_out(t, f2)
    es_9.close()
    _unscope(nc, sc9)

    es_F.close()
    es_D.close()
    es_top.close()


_CACHE = {}


def _pm(a, p=128):
    """[n*128, w] row-major -> [128, n, w] p-major contiguous."""
    n = a.shape[0] // p
    return np.ascontiguousarray(a.reshape(n, p, -1).transpose(1, 0, 2))


def _prepare(inputs):
    bf = ml_dtypes.bfloat16
    x = np.asarray(inputs["x"], dtype=np.float32)
    in_w = np.asarray(inputs["in_proj_w"], dtype=np.float32)
    out_w = np.asarray(inputs["out_proj_w"], dtype=np.float32)
    w1 = np.asarray(inputs["w1"], dtype=np.float32)
    w2 = np.asarray(inputs["w2"], dtype=np.float32)

    s1 = float(max(np.mean(np.abs(w1), dtype=np.float32), EPS))
    s2 = float(max(np.mean(np.abs(w2), dtype=np.float32), EPS))
    t1 = np.clip(np.round(w1 / np.float32(s1)), -1.0, 1.0).astype(np.float32)
    t2 = np.clip(np.round(w2 / np.float32(s2)), -1.0, 1.0).astype(np.float32)

    w1T = np.ascontiguousarray(t1.T).astype(bf)   # [D, FF]
    host = {
        "wqT": _pm(np.ascontiguousarray(in_w[0:D].T).astype(bf)),
        "wkT": _pm(np.ascontiguousarray(in_w[D:2 * D].T).astype(bf)),
        "wvT": _pm(np.ascontiguousarray(in_w[2 * D:3 * D].T).astype(bf)),
        "woT": _pm(np.ascontiguousarray(out_w.T).astype(bf)),
        # w1T chunked: [8 ffo][128 p][8 dc][512 j]
        "w1T": np.ascontiguousarray(
            w1T.reshape(DCH, 128, 8, 512).transpose(2, 1, 0, 3)),
        "w2T": _pm(np.ascontiguousarray(t2.T).astype(bf)),
    }

    def nz(a):
        return bool(np.any(np.asarray(a) != 0.0))

    biases = {
        "ln1_g": bool(np.any(np.asarray(inputs["ln1_g"]) != 1.0)),
        "ln1_b": nz(inputs["ln1_b"]),
        "ln2_g": bool(np.any(np.asarray(inputs["ln2_g"]) != 1.0)),
        "ln2_b": nz(inputs["ln2_b"]),
        "in_proj_b": nz(inputs["in_proj_b"]),
        "out_proj_b": nz(inputs["out_proj_b"]),
        "b1": nz(inputs["b1"]),
        "b2": nz(inputs["b2"]),
    }
    extra = {}
    if biases["ln1_g"]:
        extra["ln1_g"] = np.asarray(inputs["ln1_g"], np.float32)
    if biases["ln1_b"]:
        extra["ln1_b"] = np.asarray(inputs["ln1_b"], np.float32)
    if biases["ln2_g"]:
        extra["ln2_g"] = np.asarray(inputs["ln2_g"], np.float32)
    if biases["ln2_b"]:
        extra["ln2_b"] = np.asarray(inputs["ln2_b"], np.float32)
    if biases["in_proj_b"]:
        b = np.asarray(inputs["in_proj_b"], np.float32)
        extra["in_bq"] = _pm(b[0:D].reshape(D, 1))[:, :, 0]
        extra["in_bk"] = _pm(b[D:2 * D].reshape(D, 1))[:, :, 0]
        extra["in_bv"] = b[2 * D:3 * D]
    if biases["out_proj_b"]:
        extra["out_b"] = np.asarray(inputs["out_proj_b"], np.float32)
    if biases["b1"]:
        # [FF] -> [8 ffo][128 p][4 fo2]
        extra["b1"] = np.ascontiguousarray(
            np.asarray(inputs["b1"], np.float32)
            .reshape(8, 4, 128).transpose(0, 2, 1))
    if biases["b2"]:
        extra["b2"] = np.asarray(inputs["b2"], np.float32)

    x_flat = x.reshape(NTOK, D)
    in_maps = []
    for c in range(N_CORES):
        xc = x_flat[c * TOK:(c + 1) * TOK]                     # [512, 1024]
        x_pm = np.ascontiguousarray(
            xc.reshape(TCH, 128, D).transpose(1, 0, 2))        # [128, 4, 1024]
        m = {"x_pm": x_pm}
        m.update(host)
        m.update(extra)
        in_maps.append(m)
    return in_maps, s1, s2, biases


def get_program(s1, s2, biases, for_hw=True, sim_gelu=False):
    key = (round(s1, 12), round(s2, 12), tuple(sorted(biases.items())), for_hw,
           sim_gelu)
    if key not in _CACHE:
        nc = build_program(s1, s2, biases, sim_gelu=sim_gelu)
        if for_hw:
            nc.m = get_hw_module(nc.m)
        _CACHE[key] = nc
    return _CACHE[key]


def kernel(**inputs):
    in_maps, s1, s2, biases = _prepare(inputs)
    nc = get_program(s1, s2, biases, for_hw=True)
    res = run_bass_kernel_spmd(nc, in_maps, list(range(N_CORES)))
    outs = []
    for c in range(N_CORES):
        o = res.results[c]["out"]                              # [128, 4, 1024]
        outs.append(np.asarray(o).transpose(1, 0, 2).reshape(TOK, D))
    out = np.concatenate(outs, axis=0)
    return out.reshape(B, S, D).astype(np.float32)


# revision 72
# speedup vs baseline: 1.0339x; 1.0339x over previous
"""BitTransformerBlock on 8 Trainium2 NeuronCores — v4 (~450us, was 675us).

Token-parallel sharding: the flattened (B*S)=4096 tokens are split 512 per
core; cores 0-3 hold batch 0, cores 4-7 batch 1.  Each core computes LN1 and
the q/k/v projections for its own tokens, in-kernel AllGathers (replica
groups [0..3], [4..7]) share K and V across each batch group, and everything
downstream (attention over the full 2048-token context, out-proj, LN2, the
quantized FFN) is token-local.

Structure (v4):
  * the K/V exchange is TWO fp8e4m3 AllGathers (K right after the k
    projection, V after v) — ring time is linear in bytes (~25us/0.5MB
    over the 4-core group), consecutive collectives serialize, and the
    first collective pays a one-time ~15us trigger latency after the
    runtime's init barrier (which itself ends at 30-75us, jittery).
    Scores run as soon as K lands; V hides under the 4-head-pair scores
    prologue + junk matmuls.
  * the whole attention block runs in fp8e4m3: q/k/v cast to fp8 at the
    projection PSUM drain, fp8 scores matmuls (quadrant-packed head
    pairs), exp produced in fp8 (ACT exact exp for 8/16 tiles, DVE
    int8-Schraudolph bitcast for 8/16 — together they outrun the PE), and
    AV via fp8 DoubleRow matmuls contracting TWO key chunks per
    instruction.  V carries a ones column so the AV matmul emits the
    softmax denominator.
  * softmax normalization: ACT drains numerators/denominator rows (plain
    Copies, no activation tables), a PE outer-product broadcasts the
    denominator across partitions, and 1/d is a DVE bit-trick seed + one
    Newton step on the Pool engine.  (DVE's reciprocal instruction is
    free-dim-serial ~3.3us/row; GpSimd partition_broadcast is several us;
    ACT Ln<->Exp swaps cost ~1.3us per table load — all avoided.)
  * layernorm rstd = bit-trick rsqrt + Newton on [128,1] DVE tiles, so the
    only activation-table loads in the kernel are Exp (attention) and
    Gelu (ffn1).
  * ffn2 runs two passes over the w2 chunks (token halves 01 then 23,
    chunk order rotated) so the first output drains + DMAs overlap the
    second pass's matmuls.

Precision: dense PE matmuls in bf16 with fp32 PSUM accumulation; attention
in fp8e4m3 (measured ~5.3e-3 absmax-rel vs the fp32 reference, gate 2e-2).
The BitNet FFN quantization rounding is exact (magic-number round in fp32);
softmax is computed without max subtraction (logits are small here).

Known timing facts (trace-verified): PE 512-row matmul ~213ns at full
clock + 173ns SBUF access latency (hidden only by back-to-back overlap);
PE p-states 0.65/1.2/2.4GHz ramp with continuous activity (HAM re-throttles
after ~3.4us idle — hence the junk-matmul warmers); ACT/DVE exp tiles
[128,2,512] cost ~1.1/1.2us each; cross-engine chains involving DMA hops
cost ~8us latency per hop (never put them on the critical path).
"""

import numpy as np
import ml_dtypes

import concourse.bacc as bacc
import concourse.bass as bass
import concourse.bass_isa as bass_isa
import concourse.mybir as mybir
import concourse.tile as tile
from concourse import masks
from concourse.bass_interp import get_hw_module
from concourse.bass_utils import run_bass_kernel_spmd

F32 = mybir.dt.float32
BF16 = mybir.dt.bfloat16
FP8 = mybir.dt.float8e4
INT8 = mybir.dt.int8
AF = mybir.ActivationFunctionType
OP = mybir.AluOpType
DR = mybir.MatmulPerfMode.DoubleRow

N_CORES = 8
B, S, D, H, FF = 2, 2048, 1024, 16, 4096
HD = D // H                 # 64
NTOK = B * S                # 4096
TOK = NTOK // N_CORES       # 512 tokens per core
TCH = TOK // 128            # 4 token chunks per core
DCH = D // 128              # 8
FFCH = FF // 128            # 32
NKC = S // 128              # 16 key chunks per batch
GROUPS = [[0, 1, 2, 3], [4, 5, 6, 7]]
CORES_PER_B = 4
EPS = 1e-5
MAGIC = 12582912.0          # 1.5 * 2**23: fp32 round-to-nearest-even trick
INV_SQRT_HD = 1.0 / 8.0
VSLOT = HD + 1              # 65: per-head V slot width (64 dims + ones col)

# Schraudolph-style exp on DVE, fp8e4m3 flavor:
# e^(x/8) ~= bitcast_fp8e4(int8(x*EXPA8 + EXPB8)).  fp8e4m3 bits are
# [sign|4exp|3mant]; int(8*(z+7)) ~= 2^z with small sawtooth error that is
# scale-invariant under softmax and measured negligible on the final output.
EXPA8 = 8.0 / (8.0 * float(np.log(2.0)))
EXPB8 = 7.0 * 8.0 - 0.477
# which of the 16 (group, head) exp tiles per head pair go to DVE instead
# of ACT: 8/8 keeps both engines just under the PE's fp8 rate so the PE
# never stalls on softmax.  (GpSimd cannot read PSUM, so it can't help.)
EXP_DVE = {1, 3, 5, 7, 9, 11, 13, 15}
EXP_POOL = frozenset()
# fp32 reciprocal magic (Newton seed): bitcast(RCP_MAGIC - bits(x)) ~ 1/x
# with ~3.4% sawtooth error; one Newton step brings it to ~0.1%.
RCP_MAGIC = float(0x7EF127EA)
# fp32 rsqrt magic: bitcast(RSQRT_MAGIC - (bits(x) >> 1)) ~ 1/sqrt(x)
RSQRT_MAGIC = float(0x5F3759DF)
JUNK_START = 10             # junk matmuls at t=0: warm the PE before inproj
JUNK_BRIDGE = 40            # junk matmuls bridging the K-gather wait


def build_program(s1, s2, biases, sim_gelu=False):
    """Emit the SPMD program.  `s1`/`s2` are the host-computed ternary weight
    scales; `biases` maps name -> bool for whether the tensor is non-trivial."""
    nc = bacc.Bacc("TRN2", target_bir_lowering=False, debug=False,
                   num_devices=N_CORES)

    # all layouts are p-major: [128, ...] with contiguous per-partition rows
    x_in = nc.dram_tensor("x_pm", [128, TCH, D], F32, kind="ExternalInput")
    wq_in = nc.dram_tensor("wqT", [128, DCH, D], BF16, kind="ExternalInput")
    wk_in = nc.dram_tensor("wkT", [128, DCH, D], BF16, kind="ExternalInput")
    wv_in = nc.dram_tensor("wvT", [128, DCH, D], BF16, kind="ExternalInput")
    wo_in = nc.dram_tensor("woT", [128, DCH, D], BF16, kind="ExternalInput")
    w1_in = nc.dram_tensor("w1T", [8, 128, DCH, 512], BF16, kind="ExternalInput")
    w2_in = nc.dram_tensor("w2T", [128, FFCH, D], BF16, kind="ExternalInput")
    out_d = nc.dram_tensor("out", [128, TCH, D], F32, kind="ExternalOutput")

    ext = {}
    for name, shape in [("ln1_g", [D]), ("ln1_b", [D]),
                        ("ln2_g", [D]), ("ln2_b", [D]),
                        ("in_bq", [128, DCH]), ("in_bk", [128, DCH]),
                        ("in_bv", [D]), ("out_b", [D]),
                        ("b1", [128, 8, 4]), ("b2", [D])]:
        key = {"in_bq": "in_proj_b", "in_bk": "in_proj_b",
               "in_bv": "in_proj_b", "out_b": "out_proj_b"}.get(name, name)
        if biases[key]:
            ext[name] = nc.dram_tensor(name, shape, F32, kind="ExternalInput")

    with tile.TileContext(nc) as tc:
        _emit(nc, tc, x_in, wq_in, wk_in, wv_in, wo_in, w1_in, w2_in,
              out_d, ext, s1, s2, biases, sim_gelu)
    nc.compile()
    return nc


def _scope(nc, name):
    sid = nc.enter_named_scope(name, False)
    return (name, sid[0] if isinstance(sid, tuple) else sid)


def _unscope(nc, tok):
    nc.leave_named_scope(tok[0], tok[1], False)


def _emit(nc, tc, x_in, wq_in, wk_in, wv_in, wo_in, w1_in, w2_in,
          out_d, ext, s1, s2, biases, sim_gelu=False):
    gelu_func = AF.Tanh if sim_gelu else AF.Gelu
    from contextlib import ExitStack

    es_top = ExitStack()
    dram = es_top.enter_context(tc.tile_pool(name="dram", bufs=1, space="DRAM"))
    const = es_top.enter_context(tc.tile_pool(name="const", bufs=1))
    stats = es_top.enter_context(tc.tile_pool(name="stats", bufs=4))

    # split fp8 bounce buffers: kT (8*512) and v 65-slot layout (4*16*65)
    KV_K = DCH * 512                      # 4096
    KV_V = TCH * H * VSLOT               # 4160
    k_bounce = dram.tile([128, KV_K], FP8)
    k_all = dram.tile([CORES_PER_B, 128, KV_K], FP8)
    v_bounce = dram.tile([128, KV_V], FP8)
    v_all = dram.tile([CORES_PER_B, 128, KV_V], FP8)

    eps_t = const.tile([128, 1], F32)
    nc.vector.memset(eps_t[:], EPS)
    magic_t = const.tile([128, 1], F32)
    nc.vector.memset(magic_t[:], MAGIC)
    id_f32 = const.tile([128, 128], F32)
    masks.make_identity(nc, id_f32[:])
    id_bf = const.tile([128, 128], BF16)
    nc.vector.tensor_copy(out=id_bf[:], in_=id_f32[:])
    junk_sb = const.tile([128, 512], BF16, tag="junk")
    nc.gpsimd.memset(junk_sb[:], 0.001)
    ones64 = const.tile([1, 64], BF16, tag="ones64")
    nc.vector.memset(ones64[:], 1.0)

    # (no warm-up collective: with the PE pre-warmed the k bounce data is
    # ready before the init barrier ends, so the K gather pays the one-time
    # ~15us trigger latency anyway and a warm-up only adds serialization)

    # broadcast tiles for non-trivial per-feature constants (token-major use)
    def load_row_bcast(name, width):
        """DRAM [width] -> sbuf [128, width] broadcast over partitions."""
        row = const.tile([1, width], F32, tag=f"row_{name}")
        nc.sync.dma_start(out=row[:], in_=ext[name][:].unsqueeze(0))
        t = const.tile([128, width], F32, tag=f"bc_{name}")
        nc.gpsimd.partition_broadcast(t[:], row[:], channels=128)
        return t

    def load_pm(name, w=DCH):
        t = const.tile([128, w], F32, tag=f"pm_{name}")
        nc.sync.dma_start(out=t[:], in_=ext[name][:])
        return t

    g1_bc = load_row_bcast("ln1_g", D) if biases["ln1_g"] else None
    b1ln_bc = load_row_bcast("ln1_b", D) if biases["ln1_b"] else None
    bq_pm = load_pm("in_bq") if biases["in_proj_b"] else None
    bk_pm = load_pm("in_bk") if biases["in_proj_b"] else None
    bv_bc = load_row_bcast("in_bv", D) if biases["in_proj_b"] else None
    bo_bc = load_row_bcast("out_b", D) if biases["out_proj_b"] else None
    g2_bc = load_row_bcast("ln2_g", D) if biases["ln2_g"] else None
    b2ln_bc = load_row_bcast("ln2_b", D) if biases["ln2_b"] else None
    bf2_bc = load_row_bcast("b2", D) if biases["b2"] else None

    # ---- pool nesting (stack allocator: LIFO lifetimes) -------------------
    es_D = ExitStack()
    pD = es_D.enter_context(tc.tile_pool(name="pD", bufs=1))    # x2
    es_A = ExitStack()
    pA = es_A.enter_context(tc.tile_pool(name="pA", bufs=1))    # x_sb
    es_C = ExitStack()
    pC = es_C.enter_context(tc.tile_pool(name="pC", bufs=1))    # oT, wo
    es_B = ExitStack()
    pB = es_B.enter_context(tc.tile_pool(name="pB", bufs=1))    # qT, KT, Vaug
    es_X = ExitStack()
    pX = es_X.enter_context(tc.tile_pool(name="pX", bufs=1))    # xT,nxT,wqkv

    x_sb = pA.tile([128, TCH, D], F32, tag="x")
    for t in range(TCH):
        nc.sync.dma_start(out=x_sb[:, t, :], in_=x_in[:, t, :])
    nxT = pX.tile([128, DCH, TOK], BF16, tag="nxT")
    wk_sb = pX.tile([128, DCH, D], BF16, tag="wk")
    nc.sync.dma_start(out=wk_sb[:], in_=wk_in[:])
    wv_sb = pX.tile([128, DCH, D], BF16, tag="wv")
    nc.sync.dma_start(out=wv_sb[:], in_=wv_in[:])
    wq_sb = pX.tile([128, DCH, D], BF16, tag="wq")
    nc.sync.dma_start(out=wq_sb[:], in_=wq_in[:])

    es_pre = ExitStack()
    ps_pre = es_pre.enter_context(tc.tile_pool(name="ps_pre", bufs=4,
                                               space="PSUM"))
    ps_mr_pool = es_pre.enter_context(tc.tile_pool(name="ps_mr", bufs=2,
                                                   space="PSUM"))
    s_pre = es_pre.enter_context(tc.tile_pool(name="s_pre", bufs=2))

    # warm the PE (HAM clock gate) with junk matmuls while x/weights load
    junk_ps = ps_mr_pool.tile([128, 512], F32, tag="junk_ps")

    def emit_junk(n):
        for _ in range(n):
            nc.tensor.matmul(junk_ps[:], lhsT=junk_sb[:, 0:128],
                             rhs=junk_sb[:], start=True, stop=True)

    emit_junk(JUNK_START)

    # ---- stage 1: LN1 (token-major) + PE transpose to feature-major ------
    sc1 = _scope(nc, "ln1")

    def ln_stats(src_ap, m_out, r_out):
        """mean/rstd of src_ap [128, D] fp32 -> m_out/r_out [128, 1].

        rstd is a DVE bit-trick rsqrt + one Newton step (all [128,1] ops,
        ~0.1us each) — the previous exp(-0.5*ln(v)) on ACT swapped
        activation tables twice per call, ~2.5us of ACT_TABLE_LOAD."""
        st = stats.tile([128, 2, 6], F32, tag="bnst")
        nc.vector.bn_stats(out=st[:, 0, :], in_=src_ap[:, 0:512])
        nc.vector.bn_stats(out=st[:, 1, :], in_=src_ap[:, 512:1024])
        mv = stats.tile([128, 2], F32, tag="mv")
        nc.vector.bn_aggr(out=mv[:], in_=st[:])
        nc.vector.tensor_copy(out=m_out, in_=mv[:, 0:1])
        d = stats.tile([128, 1], F32, tag="vd")
        nc.vector.tensor_scalar_add(out=d[:], in0=mv[:, 1:2], scalar1=EPS)
        yi = stats.tile([128, 1], mybir.dt.int32, tag="yi")
        nc.vector.tensor_scalar(out=yi[:], in0=d[:].bitcast(mybir.dt.int32),
                                scalar1=1, scalar2=None,
                                op0=OP.logical_shift_right)
        nc.vector.tensor_scalar(out=yi[:], in0=yi[:], scalar1=-1,
                                scalar2=RSQRT_MAGIC, op0=OP.mult, op1=OP.add)
        y0 = yi[:].bitcast(F32)
        e = stats.tile([128, 1], F32, tag="e1")
        nc.vector.tensor_tensor(out=e[:], in0=d[:], in1=y0, op=OP.mult)
        nc.vector.tensor_tensor(out=e[:], in0=e[:], in1=y0, op=OP.mult)
        nc.vector.tensor_scalar(out=e[:], in0=e[:], scalar1=-0.5,
                                scalar2=1.5, op0=OP.mult, op1=OP.add)
        nc.vector.tensor_tensor(out=r_out, in0=y0, in1=e[:], op=OP.mult)

    es_t = ExitStack()
    ps_t = es_t.enter_context(tc.tile_pool(name="ps_t", bufs=2, space="PSUM"))
    for t in range(TCH):
        m = stats.tile([128, 1], F32, tag="m1")
        r = stats.tile([128, 1], F32, tag="r1")
        ln_stats(x_sb[:, t, :], m[:], r[:])
        nx = s_pre.tile([128, D], BF16, tag="nx")
        nc.vector.tensor_scalar(out=nx[:], in0=x_sb[:, t, :], scalar1=m[:],
                                scalar2=r[:], op0=OP.subtract, op1=OP.mult)
        if g1_bc is not None:
            nc.vector.tensor_tensor(out=nx[:], in0=nx[:], in1=g1_bc[:],
                                    op=OP.mult)
        if b1ln_bc is not None:
            nc.vector.tensor_tensor(out=nx[:], in0=nx[:], in1=b1ln_bc[:],
                                    op=OP.add)
        for dg in range(2):
            pst = ps_t.tile([128, 4, 128], BF16, tag="pst1")
            for i in range(4):
                dc = 4 * dg + i
                nc.tensor.transpose(pst[:, i, :],
                                    nx[:, dc * 128:(dc + 1) * 128], id_bf[:])
            for i in range(4):
                dc = 4 * dg + i
                dst = nxT[:, dc, t * 128:(t + 1) * 128]
                if i % 2 == 0:
                    nc.vector.tensor_copy(out=dst, in_=pst[:, i, :])
                else:
                    nc.scalar.activation(out=dst, in_=pst[:, i, :],
                                         func=AF.Copy)
    es_t.close()
    _unscope(nc, sc1)

    # ---- stage 2: in_proj (k -> gather, then q, then v -> gather) --------
    sc2 = _scope(nc, "inproj")
    # k projection, feature-major fp8: kT[f, t]; two fo chunks are packed
    # per bounce DMA so each transfer moves 1KB/partition
    kc = None
    for fo in range(DCH):
        ps = ps_pre.tile([128, TOK], F32, tag="ps")
        for dc in range(DCH):
            nc.tensor.matmul(ps[:], lhsT=wk_sb[:, dc, fo * 128:(fo + 1) * 128],
                             rhs=nxT[:, dc, :], start=(dc == 0),
                             stop=(dc == DCH - 1))
        if fo % 2 == 0:
            kc = s_pre.tile([128, 2, TOK], FP8, tag="kc")
        if bk_pm is not None:
            nc.vector.tensor_scalar(out=kc[:, fo % 2, :], in0=ps[:],
                                    scalar1=bk_pm[:, fo:fo + 1], op0=OP.add)
        else:
            nc.vector.tensor_copy(out=kc[:, fo % 2, :], in_=ps[:])
        if fo % 2 == 1:
            nc.sync.dma_start(
                out=k_bounce[:, (fo - 1) * 512:(fo + 1) * 512], in_=kc[:])
    nc.gpsimd.collective_compute(
        "AllGather", OP.bypass, replica_groups=GROUPS,
        ins=[k_bounce.opt()], outs=[k_all.opt()])

    # v projection, token-major fp8, written into the 65-slot bounce layout
    for t in range(TCH):
        vc = s_pre.tile([128, H, VSLOT], FP8, tag="vc")
        nc.vector.memset(vc[:, :, HD:VSLOT], 1.0)
        for f2 in range(2):
            ps = ps_pre.tile([128, 512], F32, tag="ps")
            for dc in range(DCH):
                nc.tensor.matmul(ps[:], lhsT=nxT[:, dc, t * 128:(t + 1) * 128],
                                 rhs=wv_sb[:, dc, f2 * 512:(f2 + 1) * 512],
                                 start=(dc == 0), stop=(dc == DCH - 1))
            dst = vc[:, 8 * f2:8 * (f2 + 1), 0:HD]
            if bv_bc is not None:
                nc.vector.tensor_tensor(
                    out=dst, in0=ps[:].rearrange("p (h d) -> p h d", d=HD),
                    in1=bv_bc[:, f2 * 512:(f2 + 1) * 512].rearrange(
                        "p (h d) -> p h d", d=HD), op=OP.add)
            else:
                nc.vector.tensor_copy(
                    out=dst, in_=ps[:].rearrange("p (h d) -> p h d", d=HD))
        nc.sync.dma_start(
            out=v_bounce[:, t * H * VSLOT:(t + 1) * H * VSLOT].rearrange(
                "p (h v) -> p h v", v=VSLOT), in_=vc[:])
    nc.gpsimd.collective_compute(
        "AllGather", OP.bypass, replica_groups=GROUPS,
        ins=[v_bounce.opt()], outs=[v_all.opt()])

    # q projection, feature-major fp8 (after k/v so the gathers start early)
    qT_sb = pB.tile([128, DCH, TOK], FP8, tag="qT")
    for fo in range(DCH):
        ps = ps_pre.tile([128, TOK], F32, tag="ps")
        for dc in range(DCH):
            nc.tensor.matmul(ps[:], lhsT=wq_sb[:, dc, fo * 128:(fo + 1) * 128],
                             rhs=nxT[:, dc, :], start=(dc == 0),
                             stop=(dc == DCH - 1))
        if bq_pm is not None:
            nc.vector.tensor_scalar(out=qT_sb[:, fo, :], in0=ps[:],
                                    scalar1=bq_pm[:, fo:fo + 1], op0=OP.add)
        else:
            nc.vector.tensor_copy(out=qT_sb[:, fo, :], in_=ps[:])
    _unscope(nc, sc2)

    sc3 = _scope(nc, "unpack")
    # unpack gathered K^T / V(+ones) into SBUF — pure contiguous copies.
    # KT first: scores only need K, so they can start while V still gathers.
    KT = pB.tile([128, CORES_PER_B, DCH, TOK], FP8, tag="KT")
    Vaug = pB.tile([128, CORES_PER_B, TCH, H, VSLOT], FP8, tag="Va")
    for c in range(CORES_PER_B):
        nc.sync.dma_start(
            out=KT[:, c, :, :],
            in_=k_all[c][:].rearrange("p (d t) -> p d t", t=TOK))
    for c in range(CORES_PER_B):
        nc.sync.dma_start(
            out=Vaug[:, c, :, :, :],
            in_=v_all[c][:].rearrange("p (t h v) -> p t h v", h=H, v=VSLOT))

    # prefetch wo and the first FFN weight chunks while the collectives
    # run — issued here so no later DMA queues behind the attention-era
    # denominator traffic
    wo_sb = pC.tile([128, DCH, D], BF16, tag="wo")
    nc.sync.dma_start(out=wo_sb[:], in_=wo_in[:])
    w1t = [pD.tile([128, DCH, 512], BF16, tag=f"w1_{i}", name=f"w1_{i}")
           for i in range(2)]
    nc.sync.dma_start(out=w1t[0][:], in_=w1_in[0])
    nc.sync.dma_start(out=w1t[1][:], in_=w1_in[1])
    _unscope(nc, sc3)

    es_pre.close()
    es_X.close()

    # ---- stage 3: attention (software-pipelined over head pairs) ---------
    sc4 = _scope(nc, "attn")
    oT = pC.tile([128, DCH, TOK], BF16, tag="oT")

    es_5 = ExitStack()
    ps_s = es_5.enter_context(tc.tile_pool(name="ps_s", bufs=3, space="PSUM"))
    ps_av = es_5.enter_context(tc.tile_pool(name="ps_av", bufs=2, space="PSUM"))
    s5e = es_5.enter_context(tc.tile_pool(name="s5e", bufs=30))
    s5d = es_5.enter_context(tc.tile_pool(name="s5d", bufs=3))

    NG = NKC // 2  # 8 score groups per head pair, 2 key chunks each

    def emit_scores_group(hp, g):
        """scores^T = K^T.T @ q^T for group g (row-packed head pair), + exp.

        exp tiles are spread over ACT (exact exp -> fp8), DVE and GpSimd
        (int8-Schraudolph bitcast to fp8) so the three engines together
        outrun the PE and it never stalls."""
        pss = [ps_s.tile([128, 2, 512], F32, tag="pss",
                         name=f"pss{hp}_{g}_{i}") for i in range(2)]
        for j in range(2):
            kc = 2 * g + j
            c, tcc = divmod(kc, TCH)
            ksl = KT[:, c, hp, tcc * 128:(tcc + 1) * 128]
            nc.tensor.matmul(pss[0][:, j, :], lhsT=ksl[0:64, :],
                             rhs=qT_sb[0:64, hp, :], start=True, stop=True,
                             tile_position=(0, 0))
            nc.tensor.matmul(pss[1][:, j, :], lhsT=ksl[64:128, :],
                             rhs=qT_sb[64:128, hp, :], start=True,
                             stop=True, tile_position=(64, 0))
        es = []
        for jh in range(2):
            i = 2 * g + jh
            if i in EXP_DVE or i in EXP_POOL:
                eng = nc.vector if i in EXP_DVE else nc.gpsimd
                ei = s5e.tile([128, 2, 512], INT8,
                              tag="expi" if i in EXP_DVE else "expp",
                              name=f"ei{hp}_{g}_{jh}")
                eng.tensor_scalar(out=ei[:], in0=pss[jh][:],
                                  scalar1=EXPA8, scalar2=EXPB8,
                                  op0=OP.mult, op1=OP.add)
                es.append(ei[:].bitcast(FP8))
            else:
                e = s5e.tile([128, 2, 512], FP8, tag="exp",
                             name=f"e{hp}_{g}_{jh}")
                nc.scalar.activation(out=e[:], in_=pss[jh][:], func=AF.Exp,
                                     scale=INV_SQRT_HD)
                es.append(e[:])
        return es

    AV_DOUBLE_ROW = True

    def emit_av_group(hp, g, pavs, exp_pair):
        """o^T[h] += V[2g:2g+2].T @ exp.  DoubleRow contracts both key
        chunks in one instruction but disables fast-weight-load; plain fp8
        uses two matmuls with FWL — measured A/B to pick."""
        c, tcc = divmod(2 * g, TCH)
        for jh in range(2):
            h = 2 * hp + jh
            if AV_DOUBLE_ROW:
                nc.tensor.matmul(pavs[jh][0:VSLOT, :],
                                 lhsT=Vaug[:, c, tcc:tcc + 2, h, :],
                                 rhs=exp_pair[jh],
                                 start=(g == 0), stop=(g == NG - 1),
                                 perf_mode=DR)
            else:
                for j in range(2):
                    nc.tensor.matmul(pavs[jh][0:VSLOT, :],
                                     lhsT=Vaug[:, c, tcc + j, h, :],
                                     rhs=exp_pair[jh][:, j, :],
                                     start=(g == 0 and j == 0),
                                     stop=(g == NG - 1 and j == 1))

    def finish_av_start(hp, pavs):
        """Drain the AV psum on ACT (plain Copies, no activation tables):
        numerators -> oT unnormalized, denominator rows -> bf16 staging."""
        rr = []
        for jh in range(2):
            drow = s5d.tile([1, 512], BF16, tag="drow")
            nc.scalar.activation(out=drow[:], in_=pavs[jh][64:65, :],
                                 func=AF.Copy)
            nc.scalar.activation(out=oT[jh * 64:jh * 64 + 64, hp, :],
                                 in_=pavs[jh][0:64, :], func=AF.Copy)
            rr.append(drow)
        return rr

    def finish_av_mul(hp, rr):
        """oT /= den: PE outer-product broadcasts the denominators over the
        partitions (~0.2us), DVE seeds 1/d with the bit-trick, and the
        Newton refinement runs on the otherwise-idle Pool engine."""
        dps = ps_av.tile([128, 512], F32, tag="pav", name=f"dps{hp}")
        for jh in range(2):
            nc.tensor.matmul(dps[jh * 64:jh * 64 + 64, :], lhsT=ones64[:],
                             rhs=rr[jh][:], start=True, stop=True,
                             tile_position=(0, jh * 64))
        y0 = s5d.tile([128, 512], mybir.dt.int32, tag="y0")
        nc.vector.tensor_scalar(out=y0[:], in0=dps[:].bitcast(mybir.dt.int32),
                                scalar1=-1, scalar2=RCP_MAGIC,
                                op0=OP.mult, op1=OP.add)
        y0f = y0[:].bitcast(F32)
        e2 = s5d.tile([128, 512], F32, tag="e2")
        nc.vector.tensor_tensor(out=e2[:], in0=dps[:], in1=y0f, op=OP.mult)
        nc.gpsimd.tensor_scalar(out=e2[:], in0=e2[:], scalar1=-1.0,
                                scalar2=2.0, op0=OP.mult, op1=OP.add)
        nc.gpsimd.tensor_tensor(out=oT[:, hp, :], in0=oT[:, hp, :],
                                in1=y0f, op=OP.mult)
        nc.gpsimd.tensor_tensor(out=oT[:, hp, :], in0=oT[:, hp, :],
                                in1=e2[:], op=OP.mult)

    # software pipeline: scores run three head pairs ahead of AV (the
    # depth-3 prologue covers the V-gather wait), with junk matmuls
    # topping up the PE queue until V lands
    NHP = H // 2
    DEPTH = 4

    def junk_bridge(n):
        for i in range(n):
            jt = ps_av.tile([128, 512], F32, tag="pav", name=f"junkb{i}")
            nc.tensor.matmul(jt[:], lhsT=junk_sb[:, 0:128], rhs=junk_sb[:],
                             start=True, stop=True)

    # junk first: it has no dependencies, so it fills the PE while the K
    # gather + unpack land (the prologue scores need KT)
    junk_bridge(JUNK_BRIDGE)
    exp_q = [[emit_scores_group(hp, g) for g in range(NG)]
             for hp in range(DEPTH)]

    pending = None
    for hp in range(NHP):
        # normalize of hp-1 is emitted at iteration start so the psum ring
        # rotates cleanly: pav0(h-1), pav1(h-1), dps(h-1), pav0(h), ...
        if pending is not None:
            finish_av_mul(*pending)
            pending = None
        pavs = [ps_av.tile([128, 512], F32, tag="pav", name=f"pav{hp}_{jh}")
                for jh in range(2)]
        nxt = [None] * NG
        for g in range(NG):
            if hp + DEPTH < NHP:
                nxt[g] = emit_scores_group(hp + DEPTH, g)
            emit_av_group(hp, g, pavs, exp_q[0][g])
        exp_q = exp_q[1:] + [nxt]
        pending = (hp, finish_av_start(hp, pavs))
    finish_av_mul(*pending)
    # bridge the attention->outproj seam: the final normalize chain is a
    # ~4us dependency gap (> the 3.4us HAM window), so without this the
    # out_proj matmuls start on a re-throttled clock
    junk_bridge(12)
    es_5.close()
    _unscope(nc, sc4)

    # ---- stage 4: out_proj + residual ------------------------------------
    sc6 = _scope(nc, "outproj")
    x2 = pD.tile([128, TCH, D], F32, tag="x2")
    es_6 = ExitStack()
    ps6 = es_6.enter_context(tc.tile_pool(name="ps6", bufs=4, space="PSUM"))
    for t in range(TCH):
        for f2 in range(2):
            ps = ps6.tile([128, 512], F32, tag="ps6")
            for dc in range(DCH):
                nc.tensor.matmul(ps[:], lhsT=oT[:, dc, t * 128:(t + 1) * 128],
                                 rhs=wo_sb[:, dc, f2 * 512:(f2 + 1) * 512],
                                 start=(dc == 0), stop=(dc == DCH - 1))
            dst = x2[:, t, f2 * 512:(f2 + 1) * 512]
            nc.vector.tensor_tensor(out=dst, in0=ps[:],
                                    in1=x_sb[:, t, f2 * 512:(f2 + 1) * 512],
                                    op=OP.add)
            if bo_bc is not None:
                nc.vector.tensor_tensor(out=dst, in0=dst,
                                        in1=bo_bc[:, f2 * 512:(f2 + 1) * 512],
                                        op=OP.add)
    es_6.close()
    es_B.close()
    es_C.close()
    es_A.close()
    _unscope(nc, sc6)

    # ---- stage 5: LN2 + act_quant (dq1 folded in) + PE transpose ---------
    # pF opens here (after the attention-era pools closed) so its large
    # tiles reuse the freed SBUF region
    es_F = ExitStack()
    pF = es_F.enter_context(tc.tile_pool(name="pF", bufs=1))
    sc7 = _scope(nc, "ln2q")
    hqT = pF.tile([128, DCH, TOK], BF16, tag="hqT")
    # w2 is streamed in 4 chunks of 8 fc each, double-buffered; first loads
    # issued here (they have until ffn2 to land)
    w2t = [pF.tile([128, 8, D], BF16, tag=f"w2_{i}", name=f"w2_{i}")
           for i in range(2)]
    nc.sync.dma_start(out=w2t[0][:], in_=w2_in[:, 0:8, :])
    nc.sync.dma_start(out=w2t[1][:], in_=w2_in[:, 8:16, :])

    es_7 = ExitStack()
    ps7 = es_7.enter_context(tc.tile_pool(name="ps7", bufs=2, space="PSUM"))
    s7 = es_7.enter_context(tc.tile_pool(name="s7", bufs=2))
    for t in range(TCH):
        m = stats.tile([128, 1], F32, tag="m2")
        r = stats.tile([128, 1], F32, tag="r2")
        ln_stats(x2[:, t, :], m[:], r[:])
        # h = (x2 - m) * r on ACT: Identity(x2*r + (-m*r)), freeing DVE for
        # the absmax/quant chain
        mb = stats.tile([128, 1], F32, tag="mb")
        nc.vector.tensor_tensor(out=mb[:], in0=m[:], in1=r[:], op=OP.mult)
        nc.vector.tensor_scalar_mul(out=mb[:], in0=mb[:], scalar1=-1.0)
        h = s7.tile([128, D], F32, tag="h")
        nc.scalar.activation(out=h[:], in_=x2[:, t, :], func=AF.Identity,
                             scale=r[:], bias=mb[:])
        if g2_bc is not None:
            nc.vector.tensor_tensor(out=h[:], in0=h[:], in1=g2_bc[:],
                                    op=OP.mult)
        if b2ln_bc is not None:
            nc.vector.tensor_tensor(out=h[:], in0=h[:], in1=b2ln_bc[:],
                                    op=OP.add)
        # (no EPS clamp: h is a layernorm output, absmax over 1024 unit-var
        # features is always >> EPS)
        am = stats.tile([128, 1], F32, tag="am")
        nc.vector.tensor_reduce(out=am[:], in_=h[:], axis=mybir.AxisListType.X,
                                op=OP.max, apply_absolute_value=True)
        sc = stats.tile([128, 1], F32, tag="sc")
        nc.vector.reciprocal(out=sc[:], in_=am[:])
        nc.vector.tensor_scalar_mul(out=sc[:], in0=sc[:], scalar1=127.0)
        dq1 = stats.tile([128, 1], F32, tag="dq1")
        nc.vector.tensor_scalar_mul(out=dq1[:], in0=am[:],
                                    scalar1=float(s1) / 127.0)
        # the magic-round add runs on ACT (Copy applies scale*in + bias);
        # only the subtract-and-dequant stays on DVE
        rq = s7.tile([128, D], F32, tag="rq")
        nc.scalar.activation(out=rq[:], in_=h[:], func=AF.Copy,
                             scale=sc[:], bias=float(MAGIC))
        hq = s7.tile([128, D], BF16, tag="hq")
        nc.vector.tensor_scalar(out=hq[:], in0=rq[:], scalar1=magic_t[:],
                                scalar2=dq1[:], op0=OP.subtract, op1=OP.mult)
        for dg in range(2):
            pst = ps7.tile([128, 4, 128], BF16, tag="pst")
            for i in range(4):
                dc = 4 * dg + i
                nc.tensor.transpose(pst[:, i, :],
                                    hq[:, dc * 128:(dc + 1) * 128], id_bf[:])
            for i in range(4):
                dc = 4 * dg + i
                dst = hqT[:, dc, t * 128:(t + 1) * 128]
                if i % 2 == 0:
                    nc.vector.tensor_copy(out=dst, in_=pst[:, i, :])
                else:
                    nc.scalar.activation(out=dst, in_=pst[:, i, :],
                                         func=AF.Copy)
    es_7.close()
    _unscope(nc, sc7)

    # ---- stage 6: FFN mm1 (transposed: y1T = w1 @ hqT) + gelu ------------
    sc8 = _scope(nc, "ffn1")
    y1g = pF.tile([128, FFCH, TOK], BF16, tag="y1g")
    run = pF.tile([128, TOK], F32, tag="runmax")
    nc.vector.memset(run[:], 0.0)
    if biases["b1"]:
        b1_pm = const.tile([128, 8, 4], F32, tag="pm_b1")
        nc.sync.dma_start(out=b1_pm[:], in_=ext["b1"][:])

    es_8 = ExitStack()
    ps8 = es_8.enter_context(tc.tile_pool(name="ps8", bufs=4, space="PSUM"))
    for ffo in range(8):
        wt = w1t[ffo % 2]
        for fo2 in range(4):
            fc = 4 * ffo + fo2
            ps = ps8.tile([128, TOK], F32, tag="ps8")
            for dc in range(DCH):
                nc.tensor.matmul(ps[:], lhsT=wt[:, dc, fo2 * 128:(fo2 + 1) * 128],
                                 rhs=hqT[:, dc, :], start=(dc == 0),
                                 stop=(dc == DCH - 1))
            if biases["b1"]:
                nc.scalar.activation(out=y1g[:, fc, :], in_=ps[:],
                                     func=gelu_func,
                                     bias=b1_pm[:, ffo, fo2:fo2 + 1])
            else:
                nc.scalar.activation(out=y1g[:, fc, :], in_=ps[:],
                                     func=gelu_func)
            # gelu(x) >= -0.17, and per-token max over 4096 features is
            # always >> 0.17 here, so plain max equals abs-max exactly
            nc.vector.tensor_tensor(out=run[:], in0=run[:], in1=y1g[:, fc, :],
                                    op=OP.max)
        # prefetch chunk ffo+2 into the buffer this ffo just finished reading
        if ffo + 2 < 8:
            nc.sync.dma_start(out=w1t[ffo % 2][:], in_=w1_in[ffo + 2])
    es_8.close()
    _unscope(nc, sc8)

    # ---- stage 7: act_quant of y1 (feature-major) + mm2 + residual -------
    sc9 = _scope(nc, "ffn2")
    # global per-token absmax: partition all-reduce, already broadcast
    # (no EPS clamp: the gelu absmax over 4096 features is always >> EPS)
    am_bc = pF.tile([128, TOK], F32, tag="am_bc")
    nc.gpsimd.partition_all_reduce(am_bc[:], run[:], channels=128,
                                   reduce_op=bass_isa.ReduceOp.max)
    sc_bc = pF.tile([128, TOK], F32, tag="sc_bc")
    nc.vector.reciprocal(out=sc_bc[:], in_=am_bc[:])
    nc.vector.tensor_scalar_mul(out=sc_bc[:], in0=sc_bc[:], scalar1=127.0)

    # keep the PE (and its HAM clock) busy while the absmax chain
    # (all_reduce -> reciprocal -> scale) resolves
    es_9j = ExitStack()
    ps9j = es_9j.enter_context(tc.tile_pool(name="ps9j", bufs=2, space="PSUM"))
    for i in range(30):
        jt = ps9j.tile([128, 512], F32, tag="jk", name=f"jk{i}")
        nc.tensor.matmul(jt[:], lhsT=junk_sb[:, 0:128], rhs=junk_sb[:],
                         start=True, stop=True)
    es_9j.close()

    es_9t = ExitStack()
    ps9t = es_9t.enter_context(tc.tile_pool(name="ps9t", bufs=1, space="PSUM"))
    one_t = const.tile([1, 1], F32, tag="one")
    nc.vector.memset(one_t[:], 1.0)
    dq2 = const.tile([128, TCH], F32, tag="dq2")
    pst2 = ps9t.tile([128, TCH], F32, tag="pst2")
    for t in range(TCH):
        nc.tensor.matmul(pst2[:, t:t + 1],
                         lhsT=am_bc[0:1, t * 128:(t + 1) * 128],
                         rhs=one_t[:], start=True, stop=True)
    nc.vector.tensor_scalar_mul(out=dq2[:], in0=pst2[:],
                                scalar1=float(s2) / 127.0)
    es_9t.close()

    y1qT = pF.tile([128, FFCH, TOK], BF16, tag="y1qT")
    es_9 = ExitStack()
    ps9 = es_9.enter_context(tc.tile_pool(name="ps9", bufs=8, space="PSUM"))
    s9 = es_9.enter_context(tc.tile_pool(name="s9", bufs=3))
    psf = [ps9.tile([128, 512], F32, tag="ps9", name=f"ps9_{t}_{f2}")
           for t in range(TCH) for f2 in range(2)]
    def drain_out(t, f2):
        outt = s9.tile([128, 512], F32, tag="outt")
        nc.vector.scalar_tensor_tensor(
            out=outt[:], in0=psf[2 * t + f2][:], scalar=dq2[:, t:t + 1],
            in1=x2[:, t, f2 * 512:(f2 + 1) * 512], op0=OP.mult, op1=OP.add)
        if bf2_bc is not None:
            nc.vector.tensor_tensor(
                out=outt[:], in0=outt[:],
                in1=bf2_bc[:, f2 * 512:(f2 + 1) * 512], op=OP.add)
        nc.sync.dma_start(out=out_d[:, t, f2 * 512:(f2 + 1) * 512],
                          in_=outt[:])

    # two passes over fc: token half 0/1 first (with the quant), then
    # halves 2/3 — so the first drains + output DMAs overlap the second
    # pass's matmuls instead of all landing after the last fc
    for fc in range(FFCH):
        tmp = s9.tile([128, TOK], F32, tag="qtmp")
        nc.vector.tensor_tensor(out=tmp[:], in0=y1g[:, fc, :], in1=sc_bc[:],
                                op=OP.mult)
        nc.vector.tensor_scalar(out=y1qT[:, fc, :], in0=tmp[:], scalar1=MAGIC,
                                scalar2=MAGIC, op0=OP.add, op1=OP.subtract)
        w2c, fci = divmod(fc, 8)
        for t in range(2):
            for f2 in range(2):
                nc.tensor.matmul(psf[2 * t + f2][:],
                                 lhsT=y1qT[:, fc, t * 128:(t + 1) * 128],
                                 rhs=w2t[w2c % 2][:, fci, f2 * 512:(f2 + 1) * 512],
                                 start=(fc == 0), stop=(fc == FFCH - 1))
        if fci == 7 and w2c + 2 < 4:
            nc.sync.dma_start(out=w2t[w2c % 2][:],
                              in_=w2_in[:, 8 * (w2c + 2):8 * (w2c + 3), :])
    for t in range(2):
        for f2 in range(2):
            drain_out(t, f2)
    # pass 2 consumes the still-resident w2 chunks (2, 3) first, then
    # re-streams chunks 0 and 1 into the buffers they free (fp32 psum
    # accumulation order over fc is commutative)
    seq = list(range(16, 32)) + list(range(0, 16))
    for si, fc in enumerate(seq):
        w2c, fci = divmod(fc, 8)
        for t in range(2, TCH):
            for f2 in range(2):
                nc.tensor.matmul(psf[2 * t + f2][:],
                                 lhsT=y1qT[:, fc, t * 128:(t + 1) * 128],
                                 rhs=w2t[w2c % 2][:, fci, f2 * 512:(f2 + 1) * 512],
                                 start=(si == 0), stop=(si == len(seq) - 1))
        if fci == 7 and w2c >= 2:
            nxt = w2c - 2
            nc.sync.dma_start(out=w2t[nxt % 2][:],
                              in_=w2_in[:, 8 * nxt:8 * (nxt + 1), :])
    for t in range(2, TCH):
        for f2 in range(2):
            drain_out(t, f2)
    es_9.close()
    _unscope(nc, sc9)

    es_F.close()
    es_D.close()
    es_top.close()


_CACHE = {}


def _pm(a, p=128):
    """[n*128, w] row-major -> [128, n, w] p-major contiguous."""
    n = a.shape[0] // p
    return np.ascontiguousarray(a.reshape(n, p, -1).transpose(1, 0, 2))


def _prepare(inputs):
    bf = ml_dtypes.bfloat16
    x = np.asarray(inputs["x"], dtype=np.float32)
    in_w = np.asarray(inputs["in_proj_w"], dtype=np.float32)
    out_w = np.asarray(inputs["out_proj_w"], dtype=np.float32)
    w1 = np.asarray(inputs["w1"], dtype=np.float32)
    w2 = np.asarray(inputs["w2"], dtype=np.float32)

    s1 = float(max(np.mean(np.abs(w1), dtype=np.float32), EPS))
    s2 = float(max(np.mean(np.abs(w2), dtype=np.float32), EPS))
    t1 = np.clip(np.round(w1 / np.float32(s1)), -1.0, 1.0).astype(np.float32)
    t2 = np.clip(np.round(w2 / np.float32(s2)), -1.0, 1.0).astype(np.float32)

    w1T = np.ascontiguousarray(t1.T).astype(bf)   # [D, FF]
    host = {
        "wqT": _pm(np.ascontiguousarray(in_w[0:D].T).astype(bf)),
        "wkT": _pm(np.ascontiguousarray(in_w[D:2 * D].T).astype(bf)),
        "wvT": _pm(np.ascontiguousarray(in_w[2 * D:3 * D].T).astype(bf)),
        "woT": _pm(np.ascontiguousarray(out_w.T).astype(bf)),
        # w1T chunked: [8 ffo][128 p][8 dc][512 j]
        "w1T": np.ascontiguousarray(
            w1T.reshape(DCH, 128, 8, 512).transpose(2, 1, 0, 3)),
        "w2T": _pm(np.ascontiguousarray(t2.T).astype(bf)),
    }

    def nz(a):
        return bool(np.any(np.asarray(a) != 0.0))

    biases = {
        "ln1_g": bool(np.any(np.asarray(inputs["ln1_g"]) != 1.0)),
        "ln1_b": nz(inputs["ln1_b"]),
        "ln2_g": bool(np.any(np.asarray(inputs["ln2_g"]) != 1.0)),
        "ln2_b": nz(inputs["ln2_b"]),
        "in_proj_b": nz(inputs["in_proj_b"]),
        "out_proj_b": nz(inputs["out_proj_b"]),
        "b1": nz(inputs["b1"]),
        "b2": nz(inputs["b2"]),
    }
    extra = {}
    if biases["ln1_g"]:
        extra["ln1_g"] = np.asarray(inputs["ln1_g"], np.float32)
    if biases["ln1_b"]:
        extra["ln1_b"] = np.asarray(inputs["ln1_b"], np.float32)
    if biases["ln2_g"]:
        extra["ln2_g"] = np.asarray(inputs["ln2_g"], np.float32)
    if biases["ln2_b"]:
        extra["ln2_b"] = np.asarray(inputs["ln2_b"], np.float32)
    if biases["in_proj_b"]:
        b = np.asarray(inputs["in_proj_b"], np.float32)
        extra["in_bq"] = _pm(b[0:D].reshape(D, 1))[:, :, 0]
        extra["in_bk"] = _pm(b[D:2 * D].reshape(D, 1))[:, :, 0]
        extra["in_bv"] = b[2 * D:3 * D]
    if biases["out_proj_b"]:
        extra["out_b"] = np.asarray(inputs["out_proj_b"], np.float32)
    if biases["b1"]:
        # [FF] -> [8 ffo][128 p][4 fo2]
        extra["b1"] = np.ascontiguousarray(
            np.asarray(inputs["b1"], np.float32)
            .reshape(8, 4, 128).transpose(0, 2, 1))
    if biases["b2"]:
        extra["b2"] = np.asarray(inputs["b2"], np.float32)

    x_flat = x.reshape(NTOK, D)
    in_maps = []
    for c in range(N_CORES):
        xc = x_flat[c * TOK:(c + 1) * TOK]                     # [512, 1024]
        x_pm = np.ascontiguousarray(
            xc.reshape(TCH, 128, D).transpose(1, 0, 2))        # [128, 4, 1024]
        m = {"x_pm": x_pm}
        m.update(host)
        m.update(extra)
        in_maps.append(m)
    return in_maps, s1, s2, biases


def get_program(s1, s2, biases, for_hw=True, sim_gelu=False):
    key = (round(s1, 12), round(s2, 12), tuple(sorted(biases.items())), for_hw,
           sim_gelu)
    if key not in _CACHE:
        nc = build_program(s1, s2, biases, sim_gelu=sim_gelu)
        if for_hw:
            nc.m = get_hw_module(nc.m)
        _CACHE[key] = nc
    return _CACHE[key]


def kernel(**inputs):
    in_maps, s1, s2, biases = _prepare(inputs)
    nc = get_program(s1, s2, biases, for_hw=True)
    res = run_bass_kernel_spmd(nc, in_maps, list(range(N_CORES)))
    outs = []
    for c in range(N_CORES):
        o = res.results[c]["out"]                              # [128, 4, 1024]
        outs.append(np.asarray(o).transpose(1, 0, 2).reshape(TOK, D))
    out = np.concatenate(outs, axis=0)
    return out.reshape(B, S, D).astype(np.float32)
